# revision 1
# baseline (speedup 1.0000x reference)
"""Trainium2 Bass kernel for DocAttention (doc-level CLS pairwise attention softmax).

Math (per batch b, docs x,y in [0,32), features f in [0,1024) = flattened (n,h)):
    Qf[b,x] = m[b,x] * (cls[b,x] @ Wq + bq)     cls = encoder_outputs[:,:,0,:]
    Kf[b,y] = m[b,y] * (cls[b,y] @ Wk + bk)
    att[b,n,x,y] = q.k pairwise; diag zeroed; summed over n,y
 => logits[b,x] = Qf[b,x] . (Ksum[b] - Kf[b,x]),   Ksum[b] = sum_y Kf[b,y]
    out = softmax(logits + (1-m) * -1e5)

The [B,N,A,A] attention tensor is never materialized.

Sharding over 8 NeuronCores: 4 feature-groups (256 of 1024 feature dims)
x 2 batch-halves (16 of 32 batches). Each core computes the partial
logits contribution of its 256 features for its 16 batches; the host sums
the 4 feature partials per batch-half and applies the softmax (a [32,32]
fp32 op).

The mask multiply is folded into the cls shards on the host (m*m = m for a
0/1 mask, so premasking cls makes psumQ/psumK exactly Qf/Kf; a nonzero bias
adds the rank-1 term bq (x) m via a K=1 outer-product matmul — elided when
the biases are all-zero, which they are for this problem's inputs).

All per-core inputs are packed into one DRAM tensor laid out so each
(partition, ko) slice is one contiguous ~2 KiB run: [cls 512 | csum 16 |
wq 256 | wk 256] fp16, where csum holds per-batch sums of the masked cls
rows so Ksum comes from tiny N=16 matmuls (linearity) instead of a DVE
reduce.  Eight dma_starts (one per 128-wide contraction chunk,
alternating the two HWDGE rings) keep the fixed per-DMA descriptor-gen
cost low while letting matmuls start as chunks land; dummy warmup
matmuls fill the initial DMA window so the PE clock (HAM) is at full
rate when the real stream starts.

Masked docs have all-zero premasked cls rows and contribute nothing, so
each batch's docs are compacted to nb = max(active docs per batch) slots
(zero padded; nb=21 vs 32 for this problem's mask), shrinking both DMA
bytes and the matmul moving-operand width by ~1/3. The per-feature
products Qf*(Ksum-Kf) are DMA'd straight from SBUF; the host does the
final 128-row feature reduction together with the feature-group sum, the
scatter back to full doc positions, the mask infadder, and the softmax.

Matmul dtype: fp16 (1 PE cycle/row vs 4 for fp32; halves DMA). Measured
end-to-end rel err vs the fp32 reference is ~9e-4 (logits err <~0.2 on a
scale of ~100 with min top-2 gap ~3); PSUM accumulation, the diff/product
epilogue, and the softmax are all fp32.
KERNEL_SCHEME=fp32 selects a full-fp32 build (slower, rel ~1e-6).

This walrus build encodes at most one semaphore wait per instruction;
_split_multi_waits legalizes the Tile-scheduled program by hoisting
excess waits into standalone EventSemaphore instructions.
"""

import os
import numpy as np

import concourse.bass as bass
import concourse.mybir as mybir
import concourse.tile as tile
from concourse.bass_utils import run_bass_kernel_spmd

B, A, S, D = 32, 32, 128, 1024
NH = 1024          # N*H = 16*64 flattened feature dim
P = 128
NCORES = 8
FG, BGn = 4, 2     # feature groups x batch groups
F = NH // FG       # 256 features per core
RB = B // BGn      # 16 batches per core
R = RB * A         # 512 rows (batch*doc) per core
KO = D // P        # 8 contraction chunks
FC = F // P        # 2 feature chunks of 128
PK = R + RB + 2 * F  # 1040 packed free elems per (p, ko): cls | csum | wq | wk
WARM_N = int(os.environ.get("KERNEL_WARM_N", "384"))
WARM_CNT = int(os.environ.get("KERNEL_WARM_CNT", "6"))
KSUM_MODE = os.environ.get("KERNEL_KSUM", "host")  # "host" | "dve"

SCHEME = os.environ.get("KERNEL_SCHEME", "fp16")  # "fp16" | "fp32"

_NC_CACHE = {}
LAST_RESULT = None
LAST_NB = None  # BassKernelResults of the most recent run (for test harness)


def _dt_in():
    return mybir.dt.float16 if SCHEME == "fp16" else mybir.dt.float32


def _np_dt_in():
    return np.float16 if SCHEME == "fp16" else np.float32


def _split_multi_waits(nc):
    """Hoist excess sem waits into standalone EventSemaphore instructions.

    This walrus build encodes at most one sync wait per instruction (two for
    EventSemaphore) — setupSyncWait throws "Too many sync wait commands"
    otherwise. Tile's wait assignment freely attaches several waits to one
    instruction, so split the extras into wait-only EventSemaphore
    instructions placed immediately before on the same engine (sequencers
    execute in order, so blocking there is equivalent).
    """
    n = 0
    for fn in nc.m.functions:
        for bb in fn.blocks:
            out = []
            for inst in bb.instructions:
                si = inst.sync_info
                cap = 2 if isinstance(inst, mybir.InstEventSemaphore) else 1
                if si is not None and si.on_wait and len(si.on_wait) > cap:
                    waits = list(si.on_wait)
                    extra, keep = waits[:-cap], waits[-cap:]
                    for i in range(0, len(extra), 2):
                        n += 1
                        es = mybir.InstEventSemaphore(
                            name=f"splitwait-{n}",
                            opcode="EventSemaphore",
                            engine=inst.engine,
                            sync_info=mybir.SyncInfo(
                                on_wait=extra[i : i + 2], on_update=[]
                            ),
                        )
                        nc.register_instruction(es, overwrite=True)
                        out.append(es)
                    inst.sync_info = mybir.SyncInfo(
                        on_wait=keep, on_update=list(si.on_update or [])
                    )
                out.append(inst)
            if n:
                bb.instructions = out
    return nc


def _build_nc(with_bias: bool, nb: int):
    na = RB * nb  # compacted rows per core (nb active-doc slots per batch)
    nc = bass.Bass()
    f32 = mybir.dt.float32
    dt_in = _dt_in()

    pk = na + 2 * F
    pk_d = nc.dram_tensor("pk_in", [P, KO, pk], dt_in, kind="ExternalInput")
    ks_d = nc.dram_tensor("ks_in", [P, FC, RB], f32, kind="ExternalInput")
    if with_bias:
        # [bq 256 | bk 256 | per-batch mask sums 16], plus the per-row mask
        # for the rank-1 bias terms.
        bias_d = nc.dram_tensor(
            "bias_in", [1, 2 * F + RB], dt_in, kind="ExternalInput"
        )
        m_d = nc.dram_tensor("m_in", [1, na], dt_in, kind="ExternalInput")
    out_d = nc.dram_tensor("plog", [FC * P, na], dt_in, kind="ExternalOutput")

    with tile.TileContext(nc) as tc:
        with (
            tc.tile_pool(name="const", bufs=1) as cpool,
            tc.tile_pool(name="work", bufs=int(os.environ.get("KERNEL_WBUFS", "2"))) as wpool,
            tc.tile_pool(name="psum", bufs=1, space="PSUM") as ppool,
            tc.tile_pool(name="psum_acc", bufs=1, space="PSUM") as apool,
        ):
            pk_sb = cpool.tile([P, KO, pk], dt_in)
            # PE warmup: dummy matmuls fill the DMA-wait window so the PE
            # clock (HAM) is at full rate when the real matmuls start, and
            # the ramp cost is paid off the critical path.
            warm_in = cpool.tile([P, WARM_N], dt_in)
            nc.vector.memset(warm_in, 0.0)
            ps_warm = apool.tile([P, WARM_N], mybir.dt.float32)
            for _ in range(WARM_CNT):
                nc.tensor.matmul(
                    ps_warm, lhsT=warm_in[:, 0:P], rhs=warm_in, start=True, stop=True
                )
            if with_bias:
                bias_sb = cpool.tile([1, 2 * F + RB], dt_in)
                m_sb = cpool.tile([1, na], dt_in)
                nc.sync.dma_start(out=bias_sb, in_=bias_d[:])
                nc.sync.dma_start(out=m_sb, in_=m_d[:])
            ks_sb = cpool.tile([P, FC, RB], f32)
            # one DMA per ko chunk measured fastest: every grouping that
            # enlarges the first chunk delays the stream start by more than
            # the shorter HWDGE chain saves at the tail.
            groups = [int(g) for g in os.environ.get("KERNEL_CHUNKS", "1,1,1,1,1,1,1,1").split(",")]
            assert sum(groups) == KO
            ko0 = 0
            irp = os.environ.get("KERNEL_IN_RING", "alt0")
            for gi, g in enumerate(groups):
                if irp == "alt0":
                    eng = nc.scalar if gi % 2 else nc.sync
                elif irp == "alt1":
                    eng = nc.sync if gi % 2 else nc.scalar
                elif irp == "sync":
                    eng = nc.sync
                else:
                    eng = nc.scalar
                eng.dma_start(
                    out=pk_sb[:, ko0 : ko0 + g], in_=pk_d[:][:, ko0 : ko0 + g]
                )
                ko0 += g
            # tiny exact-fp32 Ksum load; emitted after the chunk DMAs so its
            # descriptor generation doesn't delay them (needed only by the
            # DVE chain at the very end of the matmul stream)
            ksr = os.environ.get("KERNEL_KS_RING", "sync")
            ks_eng = {"scalar": nc.scalar, "sync": nc.sync}.get(ksr, nc.gpsimd)
            ks_eng.dma_start(out=ks_sb, in_=ks_d[:])

            def cls_sl(ko):
                return pk_sb[:, ko, 0:na]

            def w_sl(ko, proj, fc):
                # pack slot order [cls | wk | wq] so the last chunk can be
                # DMA'd as [cls+wk] then [wq]: the K matmuls (which gate the
                # DVE chain) start one transfer earlier.
                off = na + (F if proj == 0 else 0) + fc * P
                return pk_sb[:, ko, off : off + P]

            psq = [ppool.tile([P, na], f32, tag=f"psq{fc}", name=f"psq{fc}") for fc in range(FC)]
            psk = [ppool.tile([P, na], f32, tag=f"psk{fc}", name=f"psk{fc}") for fc in range(FC)]
            # ko-major emission so matmuls start as soon as each packed
            # chunk lands. K projections before Q in every chunk: the DVE
            # epilogue is gated on psk, and psq only needs to close before
            # the muls, two DVE ops later.
            for ko in range(KO):
                start = ko == 0
                stop = (ko == KO - 1) and not with_bias
                for proj in (1, 0):
                    for fc in range(FC):
                        nc.tensor.matmul(
                            (psq if proj == 0 else psk)[fc],
                            lhsT=w_sl(ko, proj, fc), rhs=cls_sl(ko),
                            start=start, stop=stop,
                        )
            if with_bias:
                for fc in range(FC):
                    nc.tensor.matmul(
                        psq[fc], lhsT=bias_sb[:, fc * P : (fc + 1) * P],
                        rhs=m_sb, start=False, stop=True,
                    )
                for fc in range(FC):
                    nc.tensor.matmul(
                        psk[fc], lhsT=bias_sb[:, F + fc * P : F + (fc + 1) * P],
                        rhs=m_sb, start=False, stop=True,
                    )

            # Ksum[f, b] = (sum_y cm_y) @ Wk (+ count_b * bk) is linear in
            # the inputs, tiny (16 rows, <1% of the FLOPs), and needed only
            # as a broadcast operand, so the host computes it exactly in
            # fp32 and ships it in the csum slots of the first FC packed
            # chunks; the device reads it straight from SBUF. No ksum
            # matmuls, PSUM banks, or PSUM->SBUF copies ahead of the chain.
            # prod[f, r] = Qf * (Ksum - Kf) goes straight to DRAM from
            # SBUF; the host does the cheap 128-row feature reduction while
            # summing the feature-group partials anyway; fp16 products halve
            # the writeback transfers, and fc0's writeback fixed costs
            # overlap fc1's multiply.
            for fc in range(FC):
                psk3 = psk[fc].rearrange("p (b a) -> p b a", b=RB)
                diff = wpool.tile([P, na], f32)
                diff3 = diff.rearrange("p (b a) -> p b a", b=RB)
                nc.vector.tensor_tensor(
                    diff3,
                    ks_sb[:, fc][:, :, None].to_broadcast([P, RB, nb]),
                    psk3,
                    mybir.AluOpType.subtract,
                )
                prod = wpool.tile([P, na], dt_in)
                nc.vector.tensor_mul(prod, psq[fc], diff)
                # gpsimd_sync measured fastest: the first writeback goes via
                # SWDGE (Pool) so it never occupies the shared HWDGE
                # resource, and the gating second writeback gets the SP
                # ring's shorter SEQ+DGE fixed path
                wb = os.environ.get("KERNEL_WB", "gpsimd_sync")
                if wb == "sync_scalar":
                    eng = nc.scalar if fc % 2 else nc.sync
                elif wb == "scalar_sync":
                    eng = nc.sync if fc % 2 else nc.scalar
                elif wb == "sync_sync":
                    eng = nc.sync
                elif wb == "gpsimd_sync":
                    eng = nc.sync if fc % 2 else nc.gpsimd
                else:
                    eng = nc.scalar
                eng.dma_start(out=out_d[:][fc * P : (fc + 1) * P, :], in_=prod)
    return _split_multi_waits(nc)


def _get_nc(with_bias: bool, nb: int = A):
    key = (SCHEME, with_bias, nb)
    if key not in _NC_CACHE:
        _NC_CACHE[key] = _build_nc(with_bias, nb)
    return _NC_CACHE[key]


def _prep_inputs(inputs, with_bias: bool):
    np_dt = _np_dt_in()
    enc = np.asarray(inputs["encoder_outputs"])
    mask = np.asarray(inputs["doc_attention_mask"])
    wq = np.asarray(inputs["wq"], dtype=np.float32).reshape(D, NH)
    wk = np.asarray(inputs["wk"], dtype=np.float32).reshape(D, NH)
    bq = np.asarray(inputs["bq"], dtype=np.float32).reshape(NH)
    bk = np.asarray(inputs["bk"], dtype=np.float32).reshape(NH)

    m = mask.astype(np.float32)                      # [32, 32]
    cls = np.ascontiguousarray(enc[:, :, 0, :])      # [32, 32, 1024]
    cm = cls * m[:, :, None]                         # mask folded into cls

    # Doc compaction: masked rows of cm are all-zero and contribute nothing
    # to Q, K, Ksum or the logits, so keep only nb = max(active docs per
    # batch) slots per batch (zero padded). Shrinks both the DMA bytes and
    # the matmul moving-operand width.
    counts = m.sum(axis=1).astype(int)               # [32]
    nb = max(int(counts.max()), 1)
    na = RB * nb
    active = [np.nonzero(m[b])[0] for b in range(B)]

    # clsT[bg][p, ko, r] = compacted cm rows; csums[bg] = per-batch row sums
    clsT = []
    csums = []
    for bg in range(BGn):
        half = cm[bg * RB : (bg + 1) * RB]                  # [16, 32, D]
        comp = np.zeros((RB, nb, D), np.float32)
        for j in range(RB):
            idx = active[bg * RB + j]
            comp[j, : len(idx)] = half[j, idx]
        rows = comp.reshape(na, D)
        csums.append(half.sum(axis=1))                      # [16, D] fp32
        rt = rows.T.reshape(KO, P, na).transpose(1, 0, 2)   # [P, KO, na]
        clsT.append(rt.astype(np_dt))

    wT = []  # per fg: [p, ko, 2F] = [wk 256 | wq 256]
    for fg in range(FG):
        fsl = slice(fg * F, (fg + 1) * F)
        wqt = wq[:, fsl].reshape(KO, P, F).transpose(1, 0, 2)
        wkt = wk[:, fsl].reshape(KO, P, F).transpose(1, 0, 2)
        wT.append(np.concatenate([wkt, wqt], axis=2).astype(np_dt))

    in_maps = []
    for c in range(NCORES):
        bg, fg = c // FG, c % FG
        fsl = slice(fg * F, (fg + 1) * F)
        # exact fp32 Ksum for this (half, feature group)
        ks = csums[bg] @ wk[:, fsl]                        # [16, F]
        if with_bias:
            cnt = m[bg * RB : (bg + 1) * RB].sum(axis=1)   # [RB]
            ks = ks + cnt[:, None] * bk[fsl][None, :]
        ks_in = np.zeros((P, FC, RB), np.float32)
        for fc in range(FC):
            ks_in[:, fc, :] = ks[:, fc * P : (fc + 1) * P].T
        pk = np.concatenate([clsT[bg], wT[fg]], axis=2)
        im = {"pk_in": np.ascontiguousarray(pk), "ks_in": ks_in}
        if with_bias:
            msum = m[bg * RB : (bg + 1) * RB].sum(axis=1)  # [RB]
            im["bias_in"] = np.concatenate([bq[fsl], bk[fsl], msum])[None, :].astype(
                np_dt
            )
            mcomp = np.zeros((RB, nb), np.float32)
            for j in range(RB):
                mcomp[j, : counts[bg * RB + j]] = 1.0
            im["m_in"] = mcomp.reshape(1, na).astype(np_dt)
        in_maps.append(im)
    return in_maps, m, nb, active


_FAST = {}


def _fast_run(nc, in_maps):
    """Cached-jit re-run path for repeat calls under axon.

    run_bass_kernel_spmd builds a fresh closure (and therefore a fresh
    jax.jit cache entry) per invocation; replaying the same program through
    one cached jitted shard_map skips that recompile. Mirrors
    bass2jax.run_bass_via_pjrt exactly.
    """
    import jax
    from jax.sharding import Mesh, PartitionSpec
    from jax.experimental.shard_map import shard_map
    from concourse.bass2jax import (
        _bass_exec_p,
        install_neuronx_cc_hook,
        partition_id_tensor,
    )

    key = id(nc)
    if key not in _FAST:
        install_neuronx_cc_hook()
        partition_name = (
            nc.partition_id_tensor.name if nc.partition_id_tensor else None
        )
        in_names, out_names, out_avals, zero_outs = [], [], [], []
        for alloc in nc.m.functions[0].allocations:
            if not isinstance(alloc, mybir.MemoryLocationSet):
                continue
            name = alloc.memorylocations[0].name
            if alloc.kind == "ExternalInput":
                if name != partition_name:
                    in_names.append(name)
            elif alloc.kind == "ExternalOutput":
                out_names.append(name)
                shape = tuple(alloc.tensor_shape)
                dtype = mybir.dt.np(alloc.dtype)
                out_avals.append(jax.core.ShapedArray(shape, dtype))
                zero_outs.append(np.zeros(shape, dtype))
        bind_names = in_names + out_names
        if partition_name is not None:
            bind_names = bind_names + [partition_name]

        def _body(*args):
            operands = list(args)
            if partition_name is not None:
                operands.append(partition_id_tensor())
            return tuple(
                _bass_exec_p.bind(
                    *operands,
                    out_avals=tuple(out_avals),
                    in_names=tuple(bind_names),
                    out_names=tuple(out_names),
                    lowering_input_output_aliases=(),
                    sim_require_finite=True,
                    sim_require_nnan=True,
                    nc=nc,
                )
            )

        mesh = Mesh(np.asarray(jax.devices()[:NCORES]), ("core",))
        n_args = len(in_names) + len(zero_outs)
        fn = jax.jit(
            shard_map(
                _body,
                mesh=mesh,
                in_specs=(PartitionSpec("core"),) * n_args,
                out_specs=(PartitionSpec("core"),) * len(out_names),
                check_rep=False,
            ),
            keep_unused=True,
        )
        _FAST[key] = (fn, in_names, out_names, out_avals, zero_outs)

    fn, in_names, out_names, out_avals, zero_outs = _FAST[key]
    concat_in = [
        np.concatenate([np.asarray(m[nm]) for m in in_maps], axis=0)
        for nm in in_names
    ]
    concat_zeros = [
        np.zeros((NCORES * z.shape[0], *z.shape[1:]), z.dtype) for z in zero_outs
    ]
    out_arrs = fn(*concat_in, *concat_zeros)
    return [
        {
            name: np.asarray(out_arrs[i]).reshape(NCORES, *out_avals[i].shape)[c]
            for i, name in enumerate(out_names)
        }
        for c in range(NCORES)
    ]


_CALLED = set()


def kernel(**inputs) -> np.ndarray:
    global LAST_RESULT
    with_bias = bool(
        np.any(np.asarray(inputs["bq"])) or np.any(np.asarray(inputs["bk"]))
    )
    in_maps, m, nb, active = _prep_inputs(inputs, with_bias)
    global LAST_NB
    LAST_NB = nb
    nc = _get_nc(with_bias, nb)

    from concourse._compat import axon_active

    use_fast = (
        with_bias in _CALLED
        and axon_active()
        and not os.environ.get("BASS_TRACE")
    )
    results = None
    if use_fast:
        try:
            results = _fast_run(nc, in_maps)
        except Exception:
            results = None
    if results is None:
        def _spmd():
            return run_bass_kernel_spmd(nc, in_maps, core_ids=list(range(NCORES)))

        try:
            res = _spmd()
        except ModuleNotFoundError:
            # BASS_TRACE was requested but this container lacks the axon
            # NTFF profile hook (antenv.axon_hooks); rerun without tracing.
            os.environ["BASS_NEVER_TRACE"] = "1"
            try:
                res = _spmd()
            finally:
                os.environ.pop("BASS_NEVER_TRACE", None)
        except Exception as e:  # noqa: BLE001
            # The first execution of a freshly compiled NEFF occasionally
            # reports NRT_EXEC_UNIT_UNRECOVERABLE through the axon relay;
            # an immediate retry has always succeeded. Retry once for that
            # class of failure only.
            if "UNRECOVERABLE" not in str(e) and "UNAVAILABLE" not in str(e):
                raise
            import time as _time

            res = None
            for delay in (2.0, 5.0):
                _time.sleep(delay)
                try:
                    res = _spmd()
                    break
                except Exception:  # noqa: BLE001
                    continue
            if res is None:
                _time.sleep(10.0)
                res = _spmd()
        LAST_RESULT = res
        results = res.results
    _CALLED.add(with_bias)
    na = RB * nb
    plogs = [
        r["plog"].reshape(FC * P, na).astype(np.float32).sum(axis=0)
        for r in results
    ]

    out = np.zeros((B, A), np.float32)
    for bg in range(BGn):
        plog = np.zeros(na, np.float32)
        for fg in range(FG):
            plog = plog + plogs[bg * FG + fg].astype(np.float32)
        plog = plog.reshape(RB, nb)
        # scatter compacted slots back to full doc positions (masked docs
        # keep their true logit of 0, then get the -1e5 infadder)
        logits = np.zeros((RB, A), np.float32)
        for j in range(RB):
            idx = active[bg * RB + j]
            logits[j, idx] = plog[j, : len(idx)]
        mh = m[bg * RB : (bg + 1) * RB]
        logits = logits + (1.0 - mh) * np.float32(-100000.0)
        ex = np.exp(logits - logits.max(axis=-1, keepdims=True))
        out[bg * RB : (bg + 1) * RB] = ex / ex.sum(axis=-1, keepdims=True)
    return out.astype(np.float32)

